# revision 18
# baseline (speedup 1.0000x reference)
"""Trainium2 Bass kernel for the 3-layer spiking neural network (DSNN).

Strategy (v2)
-------------
Data-parallel over batch: 256 rows / 8 cores = 32 per core, weights
replicated, zero collectives. Inside each core:

  - Layer 2 has no reset, so mem2 = (sum_t w_t out1(t)) @ W2 exactly
    (closed-form alpha/beta decay weights): one small final matmul.
  - Timestep-blocked big matmuls (Tb=16 -> 512 moving columns) keep f32r
    at 1 cyc/row and hide the ~198ns per-matmul LDWEIGHTS.
  - W0/W1/W2 all single-pass f32r (e8m11). Host-sim predicts rel-l2
    ~9.4e-3 (vs 1.9e-3 for exact-W0 hi+lo) - inside the 2e-2 gate, and
    saves ~42us of PE time.
  - The membrane recurrences of layer 0 (block k) and layer 1 (block
    k-2) run as a SINGLE fused custom DVE op per step (SNN_RESET on
    [128, 512]): state is the negated membrane; 0.0 encodes "spiked".
    Layer-1 trails layer-0 by two blocks so its drive (y1) is ready.
  - Spikes are recovered from the state by `negm == 0`:
      s0 (layer-0, feeds mm1):  DVE tensor_scalar is_eq (2x mode)
      s1 (layer-1, feeds abar): GpSimd is_eq + scalar_tensor_tensor
  - syn1 (y1) linear recurrence per block in ONE tensor_tensor_scan
    (state = alpha*state + h1) over a (lane, t) layout; a stride-Tb
    zero pattern in data0 isolates the 256 lanes from each other.
  - Membrane state ping-pongs between two tiles so the GpSimd reader
    never creates a write-after-read stall on the DVE chain.
"""

import numpy as np

ALPHA = 0.9
BETA = 0.85
THR = 1.0
T = 99            # timesteps actually simulated (t = 1..99 of 100)
BCORE = 32        # batch per core
NCORES = 8
TBM = 16          # main block size (Nk = 512 moving cols)
BLOCK_SIZES = [16, 16, 16, 16, 16, 16, 3]
assert sum(BLOCK_SIZES) == T
NB = len(BLOCK_SIZES)
TSTART = [sum(BLOCK_SIZES[:i]) for i in range(NB)]

_CACHE = {}


def _register_custom_ops():
    """SNN_RESET: m = (in0 * s0) + in1; out = m>s1 ? 0 : -m
    (negated membrane; 0.0 encodes "spiked").
    SNN_ABAR:  out = (in0 == 0) ? in1 + s0 : in1
    (weighted spike accumulation, in1/out = abar in place)."""
    import concourse.dve_ops as dve_ops
    if "SNN_RESET" in dve_ops._SUB_OPCODE_FOR_NAME:
        return (next(o for o in dve_ops.OPS if o.name == "SNN_RESET"),
                next(o for o in dve_ops.OPS if o.name == "SNN_ABAR"))
    from concourse.dve_spec import (
        Spec, Src0, Src1, Zero, select, eq, lower, _has_src1)
    from concourse.dve_uop import DveOpSpec

    def make(name, spec):
        row = dve_ops._CUSTOM_DVE_ROW_BASE + len(dve_ops.OPS)
        assert row < 0x20
        dve_ops._SUB_OPCODE_FOR_NAME[name] = row
        shas = {}
        for ver in ("v3", "v4"):
            uops = lower(spec, ver=ver)
            shas[ver] = DveOpSpec(name=name, opcode=row, uops=uops,
                                  rd1_en=_has_src1(spec)).sha(ver)
        op = dve_ops.DveOp(name, spec, subdim=False, uops_sha=shas)
        dve_ops.OPS.append(op)
        dve_ops.CUSTOM_DVE_SPECS[name] = spec
        return op

    from concourse.dve_spec import C0, C1
    f32 = np.float32
    _m = Src0 * C0 + Src1
    reset = make("SNN_RESET", Spec(
        body=select(_m > C1, Zero, Zero - _m),
        reference=lambda in0, in1, s0, s1, imm2:
            np.where((in0 * f32(s0) + in1) > f32(s1),
                     f32(0.0), -(in0 * f32(s0) + in1)).astype(f32),
    ))
    abar_op = make("SNN_ABAR", Spec(
        body=select(eq(Src0, Zero), Src1 + C0, Src1),
        reference=lambda in0, in1, s0, s1, imm2:
            np.where(in0 == 0.0, in1 + f32(s0), in1).astype(f32),
    ))
    return reset, abar_op


def _round_m11(x):
    # hw float32r = e8m11, round-to-nearest on the 12 dropped bits
    xi = np.ascontiguousarray(np.asarray(x, np.float32)).view(np.uint32).astype(np.uint64)
    bias = np.uint64(0x7FF) + ((xi >> np.uint64(12)) & np.uint64(1))
    return ((xi + bias) & np.uint64(0xFFFFF000)).astype(np.uint32).view(np.float32)


def _decay_weights():
    # w_j = sum_{k=0}^{T-1-j} BETA^(T-1-j-k) * ALPHA^k
    w = np.zeros(T, np.float64)
    for j in range(T):
        n = T - 1 - j
        k = np.arange(n + 1)
        w[j] = np.sum(BETA ** (n - k) * (ALPHA ** k))
    return w.astype(np.float32)


def build_program():
    if "nc" in _CACHE:
        return _CACHE["nc"]
    import concourse.bacc as bacc
    import concourse.mybir as mybir
    import concourse.tile as tile

    f32 = mybir.dt.float32
    f32r = mybir.dt.float32r
    A = mybir.AluOpType
    Act = mybir.ActivationFunctionType

    OP_RESET, OP_ABAR = _register_custom_ops()
    W = _decay_weights()

    nc = bacc.Bacc("TRN2", target_bir_lowering=False, debug=False,
                   enable_asserts=False, num_devices=NCORES)

    RT = nc.dram_tensor("RT", [512, T * BCORE], f32, kind="ExternalInput").ap()
    xT = nc.dram_tensor("xT", [512, BCORE], f32, kind="ExternalInput").ap()
    W0d = nc.dram_tensor("W0d", [512, 1024], f32r, kind="ExternalInput").ap()
    W1d = nc.dram_tensor("W1d", [1024, 1024], f32r, kind="ExternalInput").ap()
    W2d = nc.dram_tensor("W2d", [1024, 512], f32r, kind="ExternalInput").ap()
    b0d = nc.dram_tensor("b0d", [128, 8], f32, kind="ExternalInput").ap()
    outd = nc.dram_tensor("out", [BCORE, 512], f32, kind="ExternalOutput").ap()

    with tile.TileContext(nc) as tc:
        with (
            tc.tile_pool(name="const", bufs=1) as cpool,
            tc.tile_pool(name="rt", bufs=2) as rt_pool,
            tc.tile_pool(name="sblk", bufs=2) as s_pool,
            tc.tile_pool(name="h1p", bufs=1) as h1_pool,
            tc.tile_pool(name="s0p", bufs=2) as s0_pool,
            tc.tile_pool(name="drv", bufs=2) as drv_pool,
            tc.tile_pool(name="ps", bufs=4, space="PSUM") as ps_pool,
        ):
            # ---- constants ----
            w0_sb = cpool.tile([128, 4 * 1024], f32r, tag="w0")
            w1_sb = cpool.tile([128, 8 * 1024], f32r, tag="w1")
            b0_sb = cpool.tile([128, 8], f32, tag="b0")
            xt_sb = cpool.tile([128, 4 * BCORE], f32, tag="xt")
            # alpha pattern for the y1 scan: ALPHA everywhere except 0.0 at
            # stride-TBM positions (lane boundaries); covers 64 lanes.
            acst = cpool.tile([128, 64 * TBM], f32, tag="acst")

            nc.sync.dma_start(
                out=xt_sb[:].rearrange("p (c b) -> p c b", c=4),
                in_=xT.rearrange("(c p) b -> p c b", p=128))
            nc.sync.dma_start(out=b0_sb[:], in_=b0d)
            nc.sync.dma_start(
                out=w0_sb[:].rearrange("p (k m) -> p k m", k=4),
                in_=W0d.rearrange("(k p) m -> p k m", p=128))

            nc.vector.memset(acst[:], ALPHA)
            nc.vector.memset(
                acst[:].rearrange("p (l t) -> p l t", t=TBM)[:, :, 0:1], 0.0)

            # ---- state ----
            # negm ping-pong: [0:256) = layer-0 negm (c,b), [256:512) = layer-1
            nmA = cpool.tile([128, 512], f32, tag="nmA")
            nmB = cpool.tile([128, 512], f32, tag="nmB")
            abar = cpool.tile([128, 256], f32, tag="abar")
            zeros = cpool.tile([128, 256], f32, tag="zeros")
            ycarry = cpool.tile([128, 256], f32, tag="ycarry")
            for st in (nmA, nmB, abar, zeros):
                nc.vector.memset(st[:], 0.0)
            nm = [nmA, nmB]
            gstep = [0]

            rt4 = RT.rearrange("(c p) n -> p c n", p=128)
            rt_t, sblk_t, s0_t, drv_t = {}, {}, {}, {}
            h1 = h1_pool.tile([128, 256 * TBM], f32, tag="h1")
            h1v3 = h1[:].rearrange("p (l t) -> p l t", t=TBM)

            def stage_dma_rt(k):
                Tb = BLOCK_SIZES[k]
                Nk = Tb * BCORE
                rt = rt_pool.tile([128, 4 * TBM * BCORE], f32, tag="rt")
                for c in range(4):
                    nc.sync.dma_start(
                        out=rt[:, c * Nk:(c + 1) * Nk],
                        in_=rt4[:, c, TSTART[k] * BCORE: TSTART[k] * BCORE + Nk])
                rt_t[k] = rt

            def stage_sg(k):
                # spike-gen: compare x (broadcast over t) against rt -> f32r
                Tb = BLOCK_SIZES[k]
                Nk = Tb * BCORE
                rt = rt_t.pop(k)
                sblk = s_pool.tile([128, 4 * TBM * BCORE], f32r, tag="sblk")
                xc = (xt_sb[:].rearrange("p (c b) -> p c b", c=4)
                      .unsqueeze(2).broadcast_to([128, 4, Tb, BCORE]))
                ssl = sblk[:, :4 * Nk].rearrange("p (c t b) -> p c t b", c=4, t=Tb)
                rsl = rt[:, :4 * Nk].rearrange("p (c t b) -> p c t b", c=4, t=Tb)
                nc.vector.tensor_tensor(out=ssl, in0=xc, in1=rsl, op=A.is_gt)
                sblk_t[k] = sblk

            def stage_mm0(k):
                # H0 = S @ W0 -> drive buffer lanes [0:256), layout (c,b,t)
                Tb = BLOCK_SIZES[k]
                Nk = Tb * BCORE
                sblk = sblk_t.pop(k)
                drv = drv_t[k]
                dv = drv[:].rearrange("p (l t) -> p l t", t=TBM)
                for c in range(8):
                    ps = ps_pool.tile([128, TBM * BCORE], f32, tag="ps")
                    for ki in range(4):
                        nc.tensor.matmul(
                            ps[:, :Nk],
                            lhsT=w0_sb[:, ki * 1024 + c * 128: ki * 1024 + (c + 1) * 128],
                            rhs=sblk[:, ki * Nk:(ki + 1) * Nk],
                            start=(ki == 0), stop=(ki == 3))
                    # PSUM (t,b) -> SBUF (b,t) with bias fold
                    nc.scalar.activation(
                        out=dv[:, c * BCORE:(c + 1) * BCORE, 0:Tb],
                        in_=ps[:, :Nk].rearrange("p (t b) -> p b t", t=Tb),
                        func=Act.Identity, bias=b0_sb[:, c:c + 1], scale=1.0)

            def stage_mm1(k):
                # H1 = s0 @ W1 -> h1 buffer, layout (c,b,t)
                Tb = BLOCK_SIZES[k]
                Nk = Tb * BCORE
                s0blk = s0_t.pop(k)
                s0v = s0blk[:, :Tb * 256].rearrange("p (t l) -> p t l", t=Tb)
                for c in range(8):
                    ps = ps_pool.tile([128, TBM * BCORE], f32, tag="ps")
                    for ki in range(8):
                        nc.tensor.matmul(
                            ps[:, :Nk],
                            lhsT=w1_sb[:, ki * 1024 + c * 128: ki * 1024 + (c + 1) * 128],
                            rhs=s0v[:, :, ki * BCORE:(ki + 1) * BCORE],
                            start=(ki == 0), stop=(ki == 7))
                    nc.scalar.activation(
                        out=h1v3[:, c * BCORE:(c + 1) * BCORE, 0:Tb],
                        in_=ps[:, :Nk].rearrange("p (t b) -> p b t", t=Tb),
                        func=Act.Copy)

            def stage_scan(b1):
                # y1 recurrence for L1 block b1: y1 = ALPHA*y1 + h1, lanewise;
                # writes the y1 region (lanes [256:512)) of drive tile b1+2.
                Tb = BLOCK_SIZES[b1]
                drv = drv_t[b1 + 2]
                yreg = drv[:, 256 * TBM: 512 * TBM]
                yv = yreg.rearrange("p (l t) -> p l t", t=TBM)
                if b1 > 0:
                    # carry: h1[:, lane, 0] += ALPHA * y1_prev[lane, last]
                    nc.vector.scalar_tensor_tensor(
                        out=h1v3[:, :, 0:1], in0=ycarry[:].unsqueeze(2),
                        scalar=ALPHA, in1=h1v3[:, :, 0:1],
                        op0=A.mult, op1=A.add)
                if Tb == TBM:
                    for q in range(4):
                        sl = slice(q * 64 * TBM, (q + 1) * 64 * TBM)
                        nc.vector.tensor_tensor_scan(
                            out=yreg[:, sl], data0=acst[:], data1=h1[:, sl],
                            initial=0.0, op0=A.mult, op1=A.add)
                else:
                    for t in range(Tb):
                        if t == 0:
                            nc.vector.tensor_copy(yv[:, :, 0:1], h1v3[:, :, 0:1])
                        else:
                            nc.vector.scalar_tensor_tensor(
                                out=yv[:, :, t:t + 1], in0=yv[:, :, t - 1:t],
                                scalar=ALPHA, in1=h1v3[:, :, t:t + 1],
                                op0=A.mult, op1=A.add)
                if b1 + 1 < NB:
                    nc.vector.tensor_copy(
                        ycarry[:].unsqueeze(2), yv[:, :, Tb - 1:Tb])

            def steps(k):
                """Per-step fused recurrences for iteration k:
                L0 on block k (if k < NB), L1 on block k-2 (if k >= 2)."""
                l0 = k if k < NB else None
                l1 = k - 2 if k >= 2 else None
                n0 = BLOCK_SIZES[l0] if l0 is not None else 0
                n1 = BLOCK_SIZES[l1] if l1 is not None else 0
                drv = drv_t[k]
                dvv = drv[:].rearrange("p (l t) -> p l t", t=TBM)
                if l0 is not None:
                    s0blk = s0_pool.tile([128, TBM * 256], f32r, tag="s0")
                    s0_t[l0] = s0blk
                for t in range(max(n0, n1)):
                    do0 = l0 is not None and t < n0
                    do1 = l1 is not None and t < n1
                    p = gstep[0] % 2
                    gstep[0] += 1
                    src, dst = nm[p], nm[1 - p]
                    if do0 and do1:
                        nc.vector._custom_dve(
                            OP_RESET, out=dst[:], in0=src[:],
                            in1=dvv[:, :, t:t + 1], s0=-BETA, s1=THR)
                    elif do0:
                        nc.vector._custom_dve(
                            OP_RESET, out=dst[:, 0:256], in0=src[:, 0:256],
                            in1=dvv[:, 0:256, t:t + 1], s0=-BETA, s1=THR)
                    elif do1:
                        nc.vector._custom_dve(
                            OP_RESET, out=dst[:, 256:512], in0=src[:, 256:512],
                            in1=dvv[:, 256:512, t:t + 1], s0=-BETA, s1=THR)
                    if do0:
                        # s0 = (negm0 == 0) as f32r, t-major slot
                        nc.vector.tensor_scalar(
                            out=s0blk[:, t * 256:(t + 1) * 256],
                            in0=dst[:, 0:256], scalar1=0.0, scalar2=None,
                            op0=A.is_equal)
                    if do1:
                        # abar += w_t * (negm1 == 0), fused DVE op
                        nc.vector._custom_dve(
                            OP_ABAR, out=abar[:], in0=dst[:, 256:512],
                            in1=abar[:], s0=float(W[TSTART[l1] + t]))

            # ---------------- schedule ----------------
            stage_dma_rt(0)
            stage_dma_rt(1)
            stage_dma_rt(2)
            stage_sg(0)
            nc.sync.dma_start(
                out=w1_sb[:].rearrange("p (k m) -> p k m", k=8),
                in_=W1d.rearrange("(k p) m -> p k m", p=128))
            stage_sg(1)
            drv_t[0] = drv_pool.tile([128, 512 * TBM], f32, tag="drv",
                                     name="drv0")
            stage_mm0(0)

            for k in range(NB + 2):
                if k + 3 < NB:
                    stage_dma_rt(k + 3)
                # allocate the drive tile consumed at iteration k+1
                if k + 1 <= NB + 1:
                    drv_t[k + 1] = drv_pool.tile(
                        [128, 512 * TBM], f32, tag="drv", name=f"drv{k + 1}")
                # DVE head: y1 scan for L1 block k-2 (into drv_t[k])
                if 2 <= k:
                    stage_scan(k - 2)
                # PE: mm1 first (its h1 drain deadline is iteration k+1's scan)
                if 1 <= k <= NB:
                    stage_mm1(k - 1)
                if k + 1 < NB:
                    stage_mm0(k + 1)
                if k == 6:
                    # W2 arrives late, into the freed spike-block buffers
                    w2a = s_pool.tile([128, 4 * 512], f32r, tag="sblk",
                                      name="w2a")
                    w2b = s_pool.tile([128, 4 * 512], f32r, tag="sblk",
                                      name="w2b")
                    w2_parts = (w2a, w2b)
                    nc.sync.dma_start(
                        out=w2a[:].rearrange("p (k m) -> p k m", k=4),
                        in_=W2d.rearrange("(k p) m -> p k m", p=128)[:, 0:4])
                    nc.sync.dma_start(
                        out=w2b[:].rearrange("p (k m) -> p k m", k=4),
                        in_=W2d.rearrange("(k p) m -> p k m", p=128)[:, 4:8])
                steps(k)
                if k + 2 < NB:
                    stage_sg(k + 2)

            # ---- final: mem2 = abar @ W2 ----
            af = cpool.tile([128, 256], f32r, tag="af")
            nc.vector.tensor_copy(af[:], abar[:])
            psf = ps_pool.tile([BCORE, 512], f32, tag="psf")
            for ki in range(8):
                nc.tensor.matmul(
                    psf[:],
                    lhsT=af[:, ki * BCORE:(ki + 1) * BCORE],
                    rhs=w2_parts[ki // 4][:, (ki % 4) * 512:(ki % 4 + 1) * 512],
                    start=(ki == 0), stop=(ki == 7))
            outsb = cpool.tile([BCORE, 512], f32, tag="outsb")
            nc.scalar.activation(out=outsb[:], in_=psf[:], func=Act.Copy)
            nc.sync.dma_start(out=outd, in_=outsb[:])

    nc.compile()
    _CACHE["nc"] = nc
    return nc


def make_in_maps(inputs, W0, W1, W2, random_distribution):
    inputs = np.ascontiguousarray(np.asarray(inputs, np.float32))
    W0 = np.asarray(W0, np.float32)
    W1 = np.asarray(W1, np.float32)
    W2 = np.asarray(W2, np.float32)
    R = np.asarray(random_distribution, np.float32)

    W0r = np.ascontiguousarray(_round_m11(W0[:512]))
    W1r = np.ascontiguousarray(_round_m11(W1))
    W2r = np.ascontiguousarray(_round_m11(W2))
    b0 = np.ascontiguousarray(W0[512].reshape(8, 128).T)  # [128, 8]

    in_maps = []
    for i in range(NCORES):
        sl = slice(i * BCORE, (i + 1) * BCORE)
        xTi = np.ascontiguousarray(inputs[sl].T)  # [512, 32]
        RTi = np.ascontiguousarray(
            R[1:, sl, :512].transpose(2, 0, 1).reshape(512, T * BCORE))
        in_maps.append({
            "RT": RTi, "xT": xTi, "W0d": W0r,
            "W1d": W1r, "W2d": W2r, "b0d": b0,
        })
    return in_maps


def kernel(inputs, W0, W1, W2, random_distribution):
    from concourse.bass_utils import run_bass_kernel_spmd
    nc = build_program()
    in_maps = make_in_maps(inputs, W0, W1, W2, random_distribution)
    res = run_bass_kernel_spmd(nc, in_maps, core_ids=list(range(NCORES)))
    outs = [np.asarray(res.results[i]["out"], np.float32) for i in range(NCORES)]
    return np.concatenate(outs, axis=0)


if __name__ == "__main__":
    d = np.load("/tmp/snn_inputs.npz")
    out = kernel(d["inputs"], d["W0"], d["W1"], d["W2"], d["random_distribution"])
    exp = d["expected"]
    rel = np.linalg.norm(out - exp) / np.linalg.norm(exp)
    print("kernel vs reference rel_l2:", rel)


# revision 20
# speedup vs baseline: 1.0772x; 1.0772x over previous
"""Trainium2 Bass kernel for the 3-layer spiking neural network (DSNN).

Strategy (v3)
-------------
Data-parallel over batch: 256 rows / 8 cores = 32 per core, weights
replicated, zero collectives. Inside each core:

  - Layer 2 has no reset, so mem2 = (sum_t w_t out1(t)) @ W2 exactly
    (closed-form alpha/beta decay weights): one small final matmul.
  - The layer-1 synapse recurrence is folded into the matmul operand:
    mm1's moving tensor is the spike TRACE  strace_t = a*strace_{t-1} + s0_t,
    so  y1_t = strace_t @ W1  exactly (linearity) - no separate y1 state,
    no per-step AXPY. The trace is produced by one fused DVE op per step
    (SNN_TRACE) straight into the f32r matmul-operand slot.
  - Timestep-blocked matmuls (Tb=16 -> 512 moving columns) keep f32r at
    1 cyc/row, hide the ~200ns LDWEIGHTS, and keep the PE continuously
    busy (the PE only reaches 2.4 GHz after ~3us of uninterrupted work).
  - W0/W1/W2 single-pass f32r (e8m11): host-sim rel-l2 ~9.7e-3 vs the
    2e-2 gate; exact-W0 hi+lo would cost ~40us more PE time.
  - Membrane recurrences of layer 0 (block k) and layer 1 (block k-2)
    run as ONE fused custom DVE op per step (SNN_RESET on [128,512],
    t-major contiguous drive [h0_t | h1_t]): state is the negated
    membrane; 0.0 encodes "spiked". Layer-1 trails two blocks so its
    drive h1 = trace @ W1 is ready. Spike * w_t accumulation for the
    collapsed layer 2 is one more fused op (SNN_ABAR).
  - Membrane state ping-pongs between two tiles (nmA/nmB).

Per main step the DVE runs exactly 3 fused ops (RESET [512], TRACE
[256], ABAR [256]) ~1.5us; PE runs 3072 f32r columns ~1.3us; ScalarE
drains PSUM into the drive slots.
"""

import numpy as np

ALPHA = 0.9
BETA = 0.85
THR = 1.0
T = 99            # timesteps actually simulated (t = 1..99 of 100)
BCORE = 32        # batch per core
NCORES = 8
TBM = 16          # main block size (Nk = 512 moving cols)
BLOCK_SIZES = [16, 16, 16, 16, 16, 16, 3]
assert sum(BLOCK_SIZES) == T
NB = len(BLOCK_SIZES)
TSTART = [sum(BLOCK_SIZES[:i]) for i in range(NB)]

_CACHE = {}


def _register_custom_ops():
    """SNN_RESET: m = in0*s0 + in1; out = m>s1 ? 0 : -m   (negated membrane;
    0.0 encodes "spiked").
    SNN_TRACE: out = in1*s0 + (in0 == 0)                  (spike trace EMA).
    SNN_ABAR:  out = (in0 == 0) ? in1 + s0 : in1          (weighted spikes).
    """
    import concourse.dve_ops as dve_ops
    if "SNN_RESET" in dve_ops._SUB_OPCODE_FOR_NAME:
        return (next(o for o in dve_ops.OPS if o.name == "SNN_RESET"),
                next(o for o in dve_ops.OPS if o.name == "SNN_TRACE"),
                next(o for o in dve_ops.OPS if o.name == "SNN_ABAR"))
    from concourse.dve_spec import (
        Spec, Src0, Src1, Zero, select, eq, lower, _has_src1)
    from concourse.dve_uop import DveOpSpec

    def make(name, spec):
        row = dve_ops._CUSTOM_DVE_ROW_BASE + len(dve_ops.OPS)
        assert row < 0x20
        dve_ops._SUB_OPCODE_FOR_NAME[name] = row
        shas = {}
        for ver in ("v3", "v4"):
            uops = lower(spec, ver=ver)
            shas[ver] = DveOpSpec(name=name, opcode=row, uops=uops,
                                  rd1_en=_has_src1(spec)).sha(ver)
        op = dve_ops.DveOp(name, spec, subdim=False, uops_sha=shas)
        dve_ops.OPS.append(op)
        dve_ops.CUSTOM_DVE_SPECS[name] = spec
        return op

    from concourse.dve_spec import C0, C1
    f32 = np.float32
    _m = Src0 * C0 + Src1
    reset = make("SNN_RESET", Spec(
        body=select(_m > C1, Zero, Zero - _m),
        reference=lambda in0, in1, s0, s1, imm2:
            np.where((in0 * f32(s0) + in1) > f32(s1),
                     f32(0.0), -(in0 * f32(s0) + in1)).astype(f32),
    ))
    trace = make("SNN_TRACE", Spec(
        body=Src1 * C0 + eq(Src0, Zero),
        reference=lambda in0, in1, s0, s1, imm2:
            (in1 * f32(s0) + (in0 == 0.0)).astype(f32),
    ))
    abar_op = make("SNN_ABAR", Spec(
        body=select(eq(Src0, Zero), Src1 + C0, Src1),
        reference=lambda in0, in1, s0, s1, imm2:
            np.where(in0 == 0.0, in1 + f32(s0), in1).astype(f32),
    ))
    return reset, trace, abar_op


def _round_m11(x):
    # hw float32r = e8m11, round-to-nearest on the 12 dropped bits
    xi = np.ascontiguousarray(np.asarray(x, np.float32)).view(np.uint32).astype(np.uint64)
    bias = np.uint64(0x7FF) + ((xi >> np.uint64(12)) & np.uint64(1))
    return ((xi + bias) & np.uint64(0xFFFFF000)).astype(np.uint32).view(np.float32)


def _decay_weights():
    # w_j = sum_{k=0}^{T-1-j} BETA^(T-1-j-k) * ALPHA^k
    w = np.zeros(T, np.float64)
    for j in range(T):
        n = T - 1 - j
        k = np.arange(n + 1)
        w[j] = np.sum(BETA ** (n - k) * (ALPHA ** k))
    return w.astype(np.float32)


def build_program():
    if "nc" in _CACHE:
        return _CACHE["nc"]
    import concourse.bacc as bacc
    import concourse.mybir as mybir
    import concourse.tile as tile

    f32 = mybir.dt.float32
    f32r = mybir.dt.float32r
    A = mybir.AluOpType
    Act = mybir.ActivationFunctionType

    OP_RESET, OP_TRACE, OP_ABAR = _register_custom_ops()
    W = _decay_weights()

    nc = bacc.Bacc("TRN2", target_bir_lowering=False, debug=False,
                   enable_asserts=False, num_devices=NCORES)

    RT = nc.dram_tensor("RT", [512, T * BCORE], f32, kind="ExternalInput").ap()
    xT = nc.dram_tensor("xT", [512, BCORE], f32, kind="ExternalInput").ap()
    W0d = nc.dram_tensor("W0d", [512, 1024], f32r, kind="ExternalInput").ap()
    W1d = nc.dram_tensor("W1d", [1024, 1024], f32r, kind="ExternalInput").ap()
    W2d = nc.dram_tensor("W2d", [1024, 512], f32r, kind="ExternalInput").ap()
    b0d = nc.dram_tensor("b0d", [128, 8], f32, kind="ExternalInput").ap()
    outd = nc.dram_tensor("out", [BCORE, 512], f32, kind="ExternalOutput").ap()

    with tile.TileContext(nc) as tc:
        with (
            tc.tile_pool(name="const", bufs=1) as cpool,
            tc.tile_pool(name="rt", bufs=2) as rt_pool,
            tc.tile_pool(name="sblk", bufs=2) as s_pool,
            tc.tile_pool(name="s0p", bufs=2) as s0_pool,
            tc.tile_pool(name="drv", bufs=2) as drv_pool,
            tc.tile_pool(name="ps", bufs=4, space="PSUM") as ps_pool,
        ):
            # ---- constants ----
            w0_sb = cpool.tile([128, 4 * 1024], f32r, tag="w0")
            w1_sb = cpool.tile([128, 8 * 1024], f32r, tag="w1")
            b0_sb = cpool.tile([128, 8], f32, tag="b0")
            xt_sb = cpool.tile([128, 4 * BCORE], f32, tag="xt")

            nc.sync.dma_start(
                out=xt_sb[:].rearrange("p (c b) -> p c b", c=4),
                in_=xT.rearrange("(c p) b -> p c b", p=128))
            nc.sync.dma_start(out=b0_sb[:], in_=b0d)
            nc.sync.dma_start(
                out=w0_sb[:].rearrange("p (k m) -> p k m", k=4),
                in_=W0d.rearrange("(k p) m -> p k m", p=128))

            # ---- state ----
            # negm ping-pong: [0:256) = layer-0 negm (c,b), [256:512) = layer-1
            nmA = cpool.tile([128, 512], f32, tag="nmA")
            nmB = cpool.tile([128, 512], f32, tag="nmB")
            abar = cpool.tile([128, 256], f32, tag="abar")
            for st in (nmA, nmB, abar):
                nc.vector.memset(st[:], 0.0)
            nm = [nmA, nmB]
            gstep = [0]

            rt4 = RT.rearrange("(c p) n -> p c n", p=128)
            rt_t, sblk_t, s0_t, drv_t = {}, {}, {}, {}

            def stage_dma_rt(k):
                Tb = BLOCK_SIZES[k]
                Nk = Tb * BCORE
                rt = rt_pool.tile([128, 4 * TBM * BCORE], f32, tag="rt")
                for c in range(4):
                    nc.sync.dma_start(
                        out=rt[:, c * Nk:(c + 1) * Nk],
                        in_=rt4[:, c, TSTART[k] * BCORE: TSTART[k] * BCORE + Nk])
                rt_t[k] = rt

            def stage_sg(k):
                # spike-gen: compare x (broadcast over t) against rt -> f32r
                Tb = BLOCK_SIZES[k]
                Nk = Tb * BCORE
                rt = rt_t.pop(k)
                sblk = s_pool.tile([128, 4 * TBM * BCORE], f32r, tag="sblk")
                xc = (xt_sb[:].rearrange("p (c b) -> p c b", c=4)
                      .unsqueeze(2).broadcast_to([128, 4, Tb, BCORE]))
                ssl = sblk[:, :4 * Nk].rearrange("p (c t b) -> p c t b", c=4, t=Tb)
                rsl = rt[:, :4 * Nk].rearrange("p (c t b) -> p c t b", c=4, t=Tb)
                nc.vector.tensor_tensor(out=ssl, in0=xc, in1=rsl, op=A.is_gt)
                sblk_t[k] = sblk

            def stage_mm0(k):
                # H0 = S @ W0 -> drive tile k, slot lanes [0:256), t-major
                Tb = BLOCK_SIZES[k]
                Nk = Tb * BCORE
                sblk = sblk_t.pop(k)
                drv = drv_t[k]
                dv = drv[:].rearrange("p (t l) -> p t l", t=TBM)
                for c in range(8):
                    ps = ps_pool.tile([128, TBM * BCORE], f32, tag="ps")
                    for ki in range(4):
                        nc.tensor.matmul(
                            ps[:, :Nk],
                            lhsT=w0_sb[:, ki * 1024 + c * 128: ki * 1024 + (c + 1) * 128],
                            rhs=sblk[:, ki * Nk:(ki + 1) * Nk],
                            start=(ki == 0), stop=(ki == 3))
                    # PSUM (t,b) -> drive slots, bias fold
                    nc.scalar.activation(
                        out=dv[:, 0:Tb, c * BCORE:(c + 1) * BCORE],
                        in_=ps[:, :Nk].rearrange("p (t b) -> p t b", t=Tb),
                        func=Act.Identity, bias=b0_sb[:, c:c + 1], scale=1.0)

            def stage_mm1(k):
                # H1 = strace @ W1 -> drive tile k+2, slot lanes [256:512)
                Tb = BLOCK_SIZES[k]
                Nk = Tb * BCORE
                s0blk = s0_t[k]
                s0v = s0blk[:, :Tb * 256].rearrange("p (t l) -> p t l", t=Tb)
                drv = drv_t[k + 2]
                dv = drv[:].rearrange("p (t l) -> p t l", t=TBM)
                for c in range(8):
                    ps = ps_pool.tile([128, TBM * BCORE], f32, tag="ps")
                    for ki in range(8):
                        nc.tensor.matmul(
                            ps[:, :Nk],
                            lhsT=w1_sb[:, ki * 1024 + c * 128: ki * 1024 + (c + 1) * 128],
                            rhs=s0v[:, :, ki * BCORE:(ki + 1) * BCORE],
                            start=(ki == 0), stop=(ki == 7))
                    nc.scalar.activation(
                        out=dv[:, 0:Tb, 256 + c * BCORE:256 + (c + 1) * BCORE],
                        in_=ps[:, :Nk].rearrange("p (t b) -> p t b", t=Tb),
                        func=Act.Copy)

            def steps(k):
                """Per-step fused recurrences for iteration k:
                L0 on block k (if k < NB), L1 on block k-2 (if k >= 2)."""
                l0 = k if k < NB else None
                l1 = k - 2 if k >= 2 else None
                n0 = BLOCK_SIZES[l0] if l0 is not None else 0
                n1 = BLOCK_SIZES[l1] if l1 is not None else 0
                drv = drv_t[k]
                if l0 is not None:
                    s0blk = s0_pool.tile([128, TBM * 256], f32r, tag="s0")
                    prev_blk = s0_t.get(l0 - 1)
                    s0_t[l0] = s0blk
                for t in range(max(n0, n1)):
                    do0 = l0 is not None and t < n0
                    do1 = l1 is not None and t < n1
                    p = gstep[0] % 2
                    gstep[0] += 1
                    src, dst = nm[p], nm[1 - p]
                    slot = drv[:, t * 512:(t + 1) * 512]
                    if do0 and do1:
                        nc.vector._custom_dve(
                            OP_RESET, out=dst[:], in0=src[:],
                            in1=slot, s0=-BETA, s1=THR)
                    elif do0:
                        nc.vector._custom_dve(
                            OP_RESET, out=dst[:, 0:256], in0=src[:, 0:256],
                            in1=slot[:, 0:256], s0=-BETA, s1=THR)
                    elif do1:
                        nc.vector._custom_dve(
                            OP_RESET, out=dst[:, 256:512], in0=src[:, 256:512],
                            in1=slot[:, 256:512], s0=-BETA, s1=THR)
                    if do0:
                        # spike trace into the f32r mm1-operand slot
                        tslot = s0blk[:, t * 256:(t + 1) * 256]
                        if t > 0:
                            tprev = s0blk[:, (t - 1) * 256:t * 256]
                        elif prev_blk is not None:
                            pt = BLOCK_SIZES[l0 - 1] - 1
                            tprev = prev_blk[:, pt * 256:(pt + 1) * 256]
                        else:
                            tprev = None
                        if tprev is None:
                            nc.vector.tensor_scalar(
                                out=tslot, in0=dst[:, 0:256], scalar1=0.0,
                                scalar2=None, op0=A.is_equal)
                        else:
                            nc.vector._custom_dve(
                                OP_TRACE, out=tslot, in0=dst[:, 0:256],
                                in1=tprev, s0=ALPHA)
                    if do1:
                        # abar += w_t * (negm1 == 0), fused
                        nc.vector._custom_dve(
                            OP_ABAR, out=abar[:], in0=dst[:, 256:512],
                            in1=abar[:], s0=float(W[TSTART[l1] + t]))

            # ---------------- schedule ----------------
            stage_dma_rt(0)
            stage_dma_rt(1)
            stage_dma_rt(2)
            stage_sg(0)
            nc.sync.dma_start(
                out=w1_sb[:].rearrange("p (k m) -> p k m", k=8),
                in_=W1d.rearrange("(k p) m -> p k m", p=128))
            stage_sg(1)
            drv_t[0] = drv_pool.tile([128, 512 * TBM], f32, tag="drv",
                                     name="drv0")
            stage_mm0(0)

            for k in range(NB + 2):
                if k + 3 < NB:
                    stage_dma_rt(k + 3)
                # drive tile for iteration k+1 gets h1(k-1) and h0(k+1)
                if k + 1 <= NB + 1:
                    drv_t[k + 1] = drv_pool.tile(
                        [128, 512 * TBM], f32, tag="drv", name=f"drv{k + 1}")
                if 1 <= k <= NB:
                    stage_mm1(k - 1)
                if k + 1 < NB:
                    stage_mm0(k + 1)
                if k == 6:
                    # W2 arrives late, into the freed spike-block buffers
                    w2a = s_pool.tile([128, 4 * 512], f32r, tag="sblk",
                                      name="w2a")
                    w2b = s_pool.tile([128, 4 * 512], f32r, tag="sblk",
                                      name="w2b")
                    w2_parts = (w2a, w2b)
                    nc.sync.dma_start(
                        out=w2a[:].rearrange("p (k m) -> p k m", k=4),
                        in_=W2d.rearrange("(k p) m -> p k m", p=128)[:, 0:4])
                    nc.sync.dma_start(
                        out=w2b[:].rearrange("p (k m) -> p k m", k=4),
                        in_=W2d.rearrange("(k p) m -> p k m", p=128)[:, 4:8])
                steps(k)
                if k + 2 < NB:
                    stage_sg(k + 2)

            # ---- final: mem2 = abar @ W2 ----
            af = cpool.tile([128, 256], f32r, tag="af")
            nc.vector.tensor_copy(af[:], abar[:])
            psf = ps_pool.tile([BCORE, 512], f32, tag="psf")
            for ki in range(8):
                nc.tensor.matmul(
                    psf[:],
                    lhsT=af[:, ki * BCORE:(ki + 1) * BCORE],
                    rhs=w2_parts[ki // 4][:, (ki % 4) * 512:(ki % 4 + 1) * 512],
                    start=(ki == 0), stop=(ki == 7))
            outsb = cpool.tile([BCORE, 512], f32, tag="outsb")
            nc.scalar.activation(out=outsb[:], in_=psf[:], func=Act.Copy)
            nc.sync.dma_start(out=outd, in_=outsb[:])

    nc.compile()
    _CACHE["nc"] = nc
    return nc


def make_in_maps(inputs, W0, W1, W2, random_distribution):
    inputs = np.ascontiguousarray(np.asarray(inputs, np.float32))
    W0 = np.asarray(W0, np.float32)
    W1 = np.asarray(W1, np.float32)
    W2 = np.asarray(W2, np.float32)
    R = np.asarray(random_distribution, np.float32)

    W0r = np.ascontiguousarray(_round_m11(W0[:512]))
    W1r = np.ascontiguousarray(_round_m11(W1))
    W2r = np.ascontiguousarray(_round_m11(W2))
    b0 = np.ascontiguousarray(W0[512].reshape(8, 128).T)  # [128, 8]

    in_maps = []
    for i in range(NCORES):
        sl = slice(i * BCORE, (i + 1) * BCORE)
        xTi = np.ascontiguousarray(inputs[sl].T)  # [512, 32]
        RTi = np.ascontiguousarray(
            R[1:, sl, :512].transpose(2, 0, 1).reshape(512, T * BCORE))
        in_maps.append({
            "RT": RTi, "xT": xTi, "W0d": W0r,
            "W1d": W1r, "W2d": W2r, "b0d": b0,
        })
    return in_maps


def kernel(inputs, W0, W1, W2, random_distribution):
    from concourse.bass_utils import run_bass_kernel_spmd
    nc = build_program()
    in_maps = make_in_maps(inputs, W0, W1, W2, random_distribution)
    res = run_bass_kernel_spmd(nc, in_maps, core_ids=list(range(NCORES)))
    outs = [np.asarray(res.results[i]["out"], np.float32) for i in range(NCORES)]
    return np.concatenate(outs, axis=0)


if __name__ == "__main__":
    d = np.load("/tmp/snn_inputs.npz")
    out = kernel(d["inputs"], d["W0"], d["W1"], d["W2"], d["random_distribution"])
    exp = d["expected"]
    rel = np.linalg.norm(out - exp) / np.linalg.norm(exp)
    print("kernel vs reference rel_l2:", rel)


# revision 29
# speedup vs baseline: 1.4220x; 1.3201x over previous
"""Trainium2 Bass kernel for the 3-layer spiking neural network (DSNN).

Strategy (v3)
-------------
Data-parallel over batch: 256 rows / 8 cores = 32 per core, weights
replicated, zero collectives. Inside each core:

  - Layer 2 has no reset, so mem2 = (sum_t w_t out1(t)) @ W2 exactly
    (closed-form alpha/beta decay weights): one small final matmul.
  - The layer-1 synapse recurrence is folded into the matmul operand:
    mm1's moving tensor is the spike TRACE  strace_t = a*strace_{t-1} + s0_t,
    so  y1_t = strace_t @ W1  exactly (linearity) - no separate y1 state,
    no per-step AXPY. The trace is produced by one fused DVE op per step
    (SNN_TRACE) straight into the f32r matmul-operand slot.
  - Timestep-blocked matmuls (Tb=16 -> 512 moving columns) keep f32r at
    1 cyc/row, hide the ~200ns LDWEIGHTS, and keep the PE continuously
    busy (the PE only reaches 2.4 GHz after ~3us of uninterrupted work).
  - W0/W1/W2 single-pass f32r (e8m11): host-sim rel-l2 ~9.7e-3 vs the
    2e-2 gate; exact-W0 hi+lo would cost ~40us more PE time.
  - Membrane recurrences of layer 0 (block k) and layer 1 (block k-2)
    run as ONE fused custom DVE op per step (SNN_RESET on [128,512],
    t-major contiguous drive [h0_t | h1_t]): state is the negated
    membrane; 0.0 encodes "spiked". Layer-1 trails two blocks so its
    drive h1 = trace @ W1 is ready. Spike * w_t accumulation for the
    collapsed layer 2 is one more fused op (SNN_ABAR).
  - Membrane state ping-pongs between two tiles (nmA/nmB).

Per main step the DVE runs exactly 3 fused ops (RESET [512], TRACE
[256], ABAR [256]) ~1.5us; PE runs 3072 f32r columns ~1.3us; ScalarE
drains PSUM into the drive slots.
"""

import numpy as np

ALPHA = 0.9
BETA = 0.85
THR = 1.0
T = 99            # timesteps actually simulated (t = 1..99 of 100)
BCORE = 32        # batch per core
NCORES = 8
TBM = 16          # main block size (Nk = 512 moving cols)
BLOCK_SIZES = [8, 16, 16, 16, 16, 16, 8, 3]
assert sum(BLOCK_SIZES) == T
NB = len(BLOCK_SIZES)
TSTART = [sum(BLOCK_SIZES[:i]) for i in range(NB)]

_CACHE = {}


def _register_custom_ops():
    """SNN_RESET: m = in0*s0 + in1; out = m>s1 ? 0 : -m   (negated membrane;
    0.0 encodes "spiked").
    SNN_TRACE: out = in1*s0 + (in0 == 0)                  (spike trace EMA).
    SNN_ABAR:  out = (in0 == 0) ? in1 + s0 : in1          (weighted spikes).
    """
    import concourse.dve_ops as dve_ops
    if "SNN_RESET" in dve_ops._SUB_OPCODE_FOR_NAME:
        return (next(o for o in dve_ops.OPS if o.name == "SNN_RESET"),
                next(o for o in dve_ops.OPS if o.name == "SNN_TRACE"),
                next(o for o in dve_ops.OPS if o.name == "SNN_ABAR"))
    from concourse.dve_spec import (
        Spec, Src0, Src1, Zero, select, eq, lower, _has_src1)
    from concourse.dve_uop import DveOpSpec

    def make(name, spec):
        row = dve_ops._CUSTOM_DVE_ROW_BASE + len(dve_ops.OPS)
        assert row < 0x20
        dve_ops._SUB_OPCODE_FOR_NAME[name] = row
        shas = {}
        for ver in ("v3", "v4"):
            uops = lower(spec, ver=ver)
            shas[ver] = DveOpSpec(name=name, opcode=row, uops=uops,
                                  rd1_en=_has_src1(spec)).sha(ver)
        op = dve_ops.DveOp(name, spec, subdim=False, uops_sha=shas)
        dve_ops.OPS.append(op)
        dve_ops.CUSTOM_DVE_SPECS[name] = spec
        return op

    from concourse.dve_spec import C0, C1
    f32 = np.float32
    _m = Src0 * C0 + Src1
    reset = make("SNN_RESET", Spec(
        body=select(_m > C1, Zero, Zero - _m),
        reference=lambda in0, in1, s0, s1, imm2:
            np.where((in0 * f32(s0) + in1) > f32(s1),
                     f32(0.0), -(in0 * f32(s0) + in1)).astype(f32),
    ))
    trace = make("SNN_TRACE", Spec(
        body=Src1 * C0 + eq(Src0, Zero),
        reference=lambda in0, in1, s0, s1, imm2:
            (in1 * f32(s0) + (in0 == 0.0)).astype(f32),
    ))
    abar_op = make("SNN_ABAR", Spec(
        body=select(eq(Src0, Zero), Src1 + C0, Src1),
        reference=lambda in0, in1, s0, s1, imm2:
            np.where(in0 == 0.0, in1 + f32(s0), in1).astype(f32),
    ))
    return reset, trace, abar_op


def _round_m11(x):
    # hw float32r = e8m11, round-to-nearest on the 12 dropped bits
    xi = np.ascontiguousarray(np.asarray(x, np.float32)).view(np.uint32).astype(np.uint64)
    bias = np.uint64(0x7FF) + ((xi >> np.uint64(12)) & np.uint64(1))
    return ((xi + bias) & np.uint64(0xFFFFF000)).astype(np.uint32).view(np.float32)


def _decay_weights():
    # w_j = sum_{k=0}^{T-1-j} BETA^(T-1-j-k) * ALPHA^k
    w = np.zeros(T, np.float64)
    for j in range(T):
        n = T - 1 - j
        k = np.arange(n + 1)
        w[j] = np.sum(BETA ** (n - k) * (ALPHA ** k))
    return w.astype(np.float32)


def build_program():
    if "nc" in _CACHE:
        return _CACHE["nc"]
    import concourse.bacc as bacc
    import concourse.mybir as mybir
    import concourse.tile as tile

    f32 = mybir.dt.float32
    f32r = mybir.dt.float32r
    f16 = mybir.dt.float16
    A = mybir.AluOpType
    Act = mybir.ActivationFunctionType

    OP_RESET, OP_TRACE, OP_ABAR = _register_custom_ops()
    W = _decay_weights()

    nc = bacc.Bacc("TRN2", target_bir_lowering=False, debug=False,
                   enable_asserts=False, num_devices=NCORES)

    RT = nc.dram_tensor("RT", [512, T * BCORE], f32, kind="ExternalInput").ap()
    xT = nc.dram_tensor("xT", [512, BCORE], f32, kind="ExternalInput").ap()
    W0d = nc.dram_tensor("W0d", [512, 1024], f32r, kind="ExternalInput").ap()
    W1d = nc.dram_tensor("W1d", [1024, 1024], f16, kind="ExternalInput").ap()
    W2d = nc.dram_tensor("W2d", [1024, 512], f16, kind="ExternalInput").ap()
    b0d = nc.dram_tensor("b0d", [128, 8], f32, kind="ExternalInput").ap()
    outd = nc.dram_tensor("out", [BCORE, 512], f32, kind="ExternalOutput").ap()

    with tile.TileContext(nc) as tc:
        with (
            tc.tile_pool(name="const", bufs=1) as cpool,
            tc.tile_pool(name="rt", bufs=2) as rt_pool,
            tc.tile_pool(name="sblk", bufs=2) as s_pool,
            tc.tile_pool(name="s0p", bufs=2) as s0_pool,
            tc.tile_pool(name="drv", bufs=2) as drv_pool,
            tc.tile_pool(name="ps", bufs=4, space="PSUM") as ps_pool,
        ):
            # ---- constants ----
            w0_sb = cpool.tile([128, 4 * 1024], f32r, tag="w0")
            w1_sb = cpool.tile([128, 8 * 1024], f16, tag="w1")
            b0_sb = cpool.tile([128, 8], f32, tag="b0")
            xt_sb = cpool.tile([128, 4 * BCORE], f32, tag="xt")

            nc.sync.dma_start(
                out=xt_sb[:].rearrange("p (c b) -> p c b", c=4),
                in_=xT.rearrange("(c p) b -> p c b", p=128))
            nc.sync.dma_start(out=b0_sb[:], in_=b0d)
            nc.sync.dma_start(
                out=w0_sb[:].rearrange("p (k m) -> p k m", k=4),
                in_=W0d.rearrange("(k p) m -> p k m", p=128))

            # ---- state ----
            # negm ping-pong: [0:256) = layer-0 negm (c,b), [256:512) = layer-1
            nmA = cpool.tile([128, 512], f32, tag="nmA")
            nmB = cpool.tile([128, 512], f32, tag="nmB")
            abar = cpool.tile([128, 256], f32, tag="abar")
            for st in (nmA, nmB, abar):
                nc.vector.memset(st[:], 0.0)
            nm = [nmA, nmB]
            gstep = [0]

            rt4 = RT.rearrange("(c p) n -> p c n", p=128)
            rt_t, sblk_t, s0_t, drv_t = {}, {}, {}, {}

            def stage_dma_rt(k):
                Tb = BLOCK_SIZES[k]
                Nk = Tb * BCORE
                rt = rt_pool.tile([128, 4 * TBM * BCORE], f32, tag="rt")
                for c in range(4):
                    nc.sync.dma_start(
                        out=rt[:, c * Nk:(c + 1) * Nk],
                        in_=rt4[:, c, TSTART[k] * BCORE: TSTART[k] * BCORE + Nk])
                rt_t[k] = rt

            def stage_sg(k):
                # spike-gen: compare x (broadcast over t) against rt -> f32r
                Tb = BLOCK_SIZES[k]
                Nk = Tb * BCORE
                rt = rt_t.pop(k)
                sblk = s_pool.tile([128, 4 * TBM * BCORE], f32r, tag="sblk")
                xc = (xt_sb[:].rearrange("p (c b) -> p c b", c=4)
                      .unsqueeze(2).broadcast_to([128, 4, Tb, BCORE]))
                ssl = sblk[:, :4 * Nk].rearrange("p (c t b) -> p c t b", c=4, t=Tb)
                rsl = rt[:, :4 * Nk].rearrange("p (c t b) -> p c t b", c=4, t=Tb)
                nc.vector.tensor_tensor(out=ssl, in0=xc, in1=rsl, op=A.is_gt)
                sblk_t[k] = sblk

            def stage_mm0(k):
                # H0 = S @ W0 -> drive tile k, slot lanes [0:256), t-major
                Tb = BLOCK_SIZES[k]
                Nk = Tb * BCORE
                sblk = sblk_t.pop(k)
                drv = drv_t[k]
                dv = drv[:].rearrange("p (t l) -> p t l", t=TBM)
                for c in range(8):
                    ps = ps_pool.tile([128, TBM * BCORE], f32, tag="ps")
                    for ki in range(4):
                        nc.tensor.matmul(
                            ps[:, :Nk],
                            lhsT=w0_sb[:, ki * 1024 + c * 128: ki * 1024 + (c + 1) * 128],
                            rhs=sblk[:, ki * Nk:(ki + 1) * Nk],
                            start=(ki == 0), stop=(ki == 3))
                    # PSUM (t,b) -> drive slots, bias fold
                    nc.scalar.activation(
                        out=dv[:, 0:Tb, c * BCORE:(c + 1) * BCORE],
                        in_=ps[:, :Nk].rearrange("p (t b) -> p t b", t=Tb),
                        func=Act.Identity, bias=b0_sb[:, c:c + 1], scale=1.0)

            def stage_mm1(k):
                # H1 = strace @ W1 -> drive tile k+2, slot lanes [256:512)
                Tb = BLOCK_SIZES[k]
                Nk = Tb * BCORE
                s0blk = s0_t[k]
                s0v = s0blk[:, :Tb * 256].rearrange("p (t l) -> p t l", t=Tb)
                drv = drv_t[k + 2]
                dv = drv[:].rearrange("p (t l) -> p t l", t=TBM)
                for c in range(8):
                    ps = ps_pool.tile([128, TBM * BCORE], f32, tag="ps")
                    for ki in range(8):
                        nc.tensor.matmul(
                            ps[:, :Nk],
                            lhsT=w1_sb[:, ki * 1024 + c * 128: ki * 1024 + (c + 1) * 128],
                            rhs=s0v[:, :, ki * BCORE:(ki + 1) * BCORE],
                            start=(ki == 0), stop=(ki == 7))
                    nc.scalar.activation(
                        out=dv[:, 0:Tb, 256 + c * BCORE:256 + (c + 1) * BCORE],
                        in_=ps[:, :Nk].rearrange("p (t b) -> p t b", t=Tb),
                        func=Act.Copy)

            def steps(k):
                """Per-step fused recurrences for iteration k:
                L0 on block k (if k < NB), L1 on block k-2 (if k >= 2)."""
                l0 = k if k < NB else None
                l1 = k - 2 if k >= 2 else None
                n0 = BLOCK_SIZES[l0] if l0 is not None else 0
                n1 = BLOCK_SIZES[l1] if l1 is not None else 0
                drv = drv_t[k]
                if l0 is not None:
                    s0blk = s0_pool.tile([128, TBM * 256], f16, tag="s0")
                    prev_blk = s0_t.get(l0 - 1)
                    s0_t[l0] = s0blk
                for t in range(max(n0, n1)):
                    do0 = l0 is not None and t < n0
                    do1 = l1 is not None and t < n1
                    p = gstep[0] % 2
                    gstep[0] += 1
                    src, dst = nm[p], nm[1 - p]
                    slot = drv[:, t * 512:(t + 1) * 512]
                    if do0 and do1:
                        nc.vector._custom_dve(
                            OP_RESET, out=dst[:], in0=src[:],
                            in1=slot, s0=-BETA, s1=THR)
                    elif do0:
                        nc.vector._custom_dve(
                            OP_RESET, out=dst[:, 0:256], in0=src[:, 0:256],
                            in1=slot[:, 0:256], s0=-BETA, s1=THR)
                    elif do1:
                        nc.vector._custom_dve(
                            OP_RESET, out=dst[:, 256:512], in0=src[:, 256:512],
                            in1=slot[:, 256:512], s0=-BETA, s1=THR)
                    if do1:
                        # abar += w_t * (negm1 == 0), fused
                        nc.vector._custom_dve(
                            OP_ABAR, out=abar[:], in0=dst[:, 256:512],
                            in1=abar[:], s0=float(W[TSTART[l1] + t]))
                    if do0:
                        # spike trace into the fp16 mm1-operand slot
                        tslot = s0blk[:, t * 256:(t + 1) * 256]
                        if t > 0:
                            tprev = s0blk[:, (t - 1) * 256:t * 256]
                        elif prev_blk is not None:
                            pt = BLOCK_SIZES[l0 - 1] - 1
                            tprev = prev_blk[:, pt * 256:(pt + 1) * 256]
                        else:
                            tprev = None
                        if tprev is None:
                            nc.vector.tensor_scalar(
                                out=tslot, in0=dst[:, 0:256], scalar1=0.0,
                                scalar2=None, op0=A.is_equal)
                        else:
                            nc.vector._custom_dve(
                                OP_TRACE, out=tslot, in0=dst[:, 0:256],
                                in1=tprev, s0=ALPHA)

            # ---------------- schedule ----------------
            stage_dma_rt(0)
            stage_dma_rt(1)
            stage_dma_rt(2)
            stage_sg(0)
            nc.sync.dma_start(
                out=w1_sb[:].rearrange("p (k m) -> p k m", k=8),
                in_=W1d.rearrange("(k p) m -> p k m", p=128))
            stage_sg(1)
            drv_t[0] = drv_pool.tile([128, 512 * TBM], f32, tag="drv",
                                     name="drv0")
            stage_mm0(0)

            for k in range(NB + 2):
                if k + 3 < NB:
                    stage_dma_rt(k + 3)
                # drive tile for iteration k+1 gets h1(k-1) and h0(k+1)
                if k + 1 <= NB + 1:
                    drv_t[k + 1] = drv_pool.tile(
                        [128, 512 * TBM], f32, tag="drv", name=f"drv{k + 1}")
                if 1 <= k <= NB:
                    stage_mm1(k - 1)
                if k + 1 < NB:
                    stage_mm0(k + 1)
                if k == NB - 1:
                    # W2 (fp16) arrives late, into a freed spike-block buffer
                    w2_sb = s_pool.tile([128, 8 * 512], f16, tag="sblk",
                                        name="w2_sb")
                    nc.sync.dma_start(
                        out=w2_sb[:].rearrange("p (k m) -> p k m", k=8),
                        in_=W2d.rearrange("(k p) m -> p k m", p=128))
                steps(k)
                if k + 2 < NB:
                    stage_sg(k + 2)

            # ---- final: mem2 = abar @ W2 ----
            af = cpool.tile([128, 256], f16, tag="af")
            nc.vector.tensor_copy(af[:], abar[:])
            psf = ps_pool.tile([BCORE, 512], f32, tag="psf")
            for ki in range(8):
                nc.tensor.matmul(
                    psf[:],
                    lhsT=af[:, ki * BCORE:(ki + 1) * BCORE],
                    rhs=w2_sb[:, ki * 512:(ki + 1) * 512],
                    start=(ki == 0), stop=(ki == 7))
            outsb = cpool.tile([BCORE, 512], f32, tag="outsb")
            nc.scalar.activation(out=outsb[:], in_=psf[:], func=Act.Copy)
            nc.sync.dma_start(out=outd, in_=outsb[:])

    nc.compile()
    _CACHE["nc"] = nc
    return nc


def make_in_maps(inputs, W0, W1, W2, random_distribution):
    inputs = np.ascontiguousarray(np.asarray(inputs, np.float32))
    W0 = np.asarray(W0, np.float32)
    W1 = np.asarray(W1, np.float32)
    W2 = np.asarray(W2, np.float32)
    R = np.asarray(random_distribution, np.float32)

    W0r = np.ascontiguousarray(_round_m11(W0[:512]))
    W1r = np.ascontiguousarray(W1.astype(np.float16))
    W2r = np.ascontiguousarray(W2.astype(np.float16))
    b0 = np.ascontiguousarray(W0[512].reshape(8, 128).T)  # [128, 8]

    in_maps = []
    for i in range(NCORES):
        sl = slice(i * BCORE, (i + 1) * BCORE)
        xTi = np.ascontiguousarray(inputs[sl].T)  # [512, 32]
        RTi = np.ascontiguousarray(
            R[1:, sl, :512].transpose(2, 0, 1).reshape(512, T * BCORE))
        in_maps.append({
            "RT": RTi, "xT": xTi, "W0d": W0r,
            "W1d": W1r, "W2d": W2r, "b0d": b0,
        })
    return in_maps


def kernel(inputs, W0, W1, W2, random_distribution):
    from concourse.bass_utils import run_bass_kernel_spmd
    nc = build_program()
    in_maps = make_in_maps(inputs, W0, W1, W2, random_distribution)
    res = run_bass_kernel_spmd(nc, in_maps, core_ids=list(range(NCORES)))
    outs = [np.asarray(res.results[i]["out"], np.float32) for i in range(NCORES)]
    return np.concatenate(outs, axis=0)


if __name__ == "__main__":
    d = np.load("/tmp/snn_inputs.npz")
    out = kernel(d["inputs"], d["W0"], d["W1"], d["W2"], d["random_distribution"])
    exp = d["expected"]
    rel = np.linalg.norm(out - exp) / np.linalg.norm(exp)
    print("kernel vs reference rel_l2:", rel)
